# revision 8
# baseline (speedup 1.0000x reference)
"""Single-head causal attention (B=8, S=2048, E=1024, D=64) on 8 TRN2 cores.

Strategy: pure data parallelism over batch (8 batch elements -> 8 cores, no
collectives). Each core computes q/k/v projections, causal softmax(q k^T/8),
the full attention-probability matrix (an output), and attn @ v.

Per-core dataflow (matmul operands bf16 for full PE rate; accumulation and
both outputs stay f32):
  - host passes xT = x[b].T in bf16 so the contraction dim (embd) lands on
    SBUF partitions without any on-device transpose
  - qT/kT computed in one pass with packed [W_q|W_k] stationary (M=128);
    vT in a second pass; v tiles recovered with PE transposes
  - scores per q-tile in [q, k] layout -> causal bias add on the diagonal
    block -> one fused Exp (+row-sum via accum_out) -> reciprocal ->
    normalize -> DMA to attn. The strict upper triangle is never written:
    output buffers arrive pre-zeroed, and exp(-1e10/8) == 0 exactly.
  - scores recomputed per k-tile in [k, q] layout (cheaper than transposing
    136 tiles + evicting), exp'd to bf16, and fed as the moving operand to
    v-stationary matmuls accumulating outT [64, 2048] in PSUM
  - outT is PE-transposed back to [2048, 64] and scaled by the row
    reciprocals on eviction.
"""

import numpy as np

BATCH = 8
SEQ = 2048
EMB = 1024
D = 64
P = 128
BK = 512  # matmul moving-operand block (one f32 PSUM bank)
NT = SEQ // P  # 16 row/col tiles
NB = SEQ // BK  # 4 column blocks
EC = EMB // P  # 8 embedding chunks
MASK_NEG = -1.0e10  # exp(0.125 * -1e10) == 0.0 exactly in fp32

_programs: dict = {}


def _build_program(causal: bool, repeat: int = 1):
    from contextlib import ExitStack

    import concourse.bacc as bacc
    import concourse.mybir as mybir
    import concourse.tile as tile
    from concourse.masks import make_causal_mask, make_identity, make_lower_triangular

    f32 = mybir.dt.float32
    bf16 = mybir.dt.bfloat16
    Exp = mybir.ActivationFunctionType.Exp
    add = mybir.AluOpType.add

    nc = bacc.Bacc("TRN2", target_bir_lowering=False, debug=False)

    xT = nc.dram_tensor("xT", [EMB, SEQ], bf16, kind="ExternalInput")
    wqk = nc.dram_tensor("wqk", [EMB, 2 * D], bf16, kind="ExternalInput")
    wv = nc.dram_tensor("wv", [EMB, D], bf16, kind="ExternalInput")
    attn = nc.dram_tensor("attn", [SEQ, SEQ], f32, kind="ExternalOutput")
    out = nc.dram_tensor("out", [SEQ, D], f32, kind="ExternalOutput")

    with tile.TileContext(nc) as tc, ExitStack() as ctx:
        consts = ctx.enter_context(tc.tile_pool(name="consts", bufs=1))
        acts = ctx.enter_context(tc.tile_pool(name="acts", bufs=1))
        epool = ctx.enter_context(tc.tile_pool(name="ep", bufs=3))
        etpool = ctx.enter_context(tc.tile_pool(name="etp", bufs=2))
        opool = ctx.enter_context(tc.tile_pool(name="op", bufs=3))

        ident_b = consts.tile([P, P], bf16)
        make_identity(nc, ident_b)
        ident_f = consts.tile([P, P], f32)
        make_identity(nc, ident_f)
        if causal:
            # [q, k] diagonal-block bias: 0 where k <= q, -1e10 above
            cmask = consts.tile([P, P], f32)
            make_causal_mask(nc, cmask, mask_val=MASK_NEG)
            # [k, q] diagonal-block bias: 0 where q >= k, -1e10 strictly below
            cmaskT = consts.tile([P, P], f32)
            make_lower_triangular(nc, cmaskT, val=MASK_NEG, diag=False)

        wqk_sb = consts.tile([P, EC, 2 * D], bf16)
        nc.sync.dma_start(wqk_sb[:], wqk.rearrange("(o p) m -> p o m", p=P))
        wv_sb = consts.tile([P, EC, D], bf16)
        nc.sync.dma_start(wv_sb[:], wv.rearrange("(o p) m -> p o m", p=P))

        xT_sb = acts.tile([P, EC, SEQ], bf16)
        xT_r = xT.rearrange("(o p) s -> p o s", p=P)
        for c in range(NB):
            cs = slice(c * BK, (c + 1) * BK)
            nc.sync.dma_start(xT_sb[:, :, cs], xT_r[:, :, cs])

        for _rep in range(repeat):
            _kernel_body(nc, tc, mybir, causal, locals())

    nc.finalize()
    return nc


def _kernel_body(nc, tc, mybir, causal, env):
    from contextlib import ExitStack

    f32 = mybir.dt.float32
    bf16 = mybir.dt.bfloat16
    Exp = mybir.ActivationFunctionType.Exp
    add = mybir.AluOpType.add
    acts = env["acts"]
    epool, etpool, opool = env["ep" + "ool"], env["etpool"], env["opool"]
    ident_b, ident_f = env["ident_b"], env["ident_f"]
    cmask = env.get("cmask")
    cmaskT = env.get("cmaskT")
    wqk_sb, wv_sb, xT_sb = env["wqk_sb"], env["wv_sb"], env["xT_sb"]
    attn, out = env["attn"], env["out"]
    if True:
        qkT_sb = acts.tile([P, SEQ], bf16, tag="qkT")  # rows 0:64 qT, 64:128 kT
        kT_sb = acts.tile([D, SEQ], bf16, tag="kT")
        vT_sb = acts.tile([D, SEQ], bf16, tag="vT")
        v_sb = acts.tile([P, NT, D], bf16, tag="v")
        outT_sb = acts.tile([D, SEQ], f32, tag="outT")
        recip = acts.tile([P, NT], f32, tag="recip")

        # ---- Phase 2: projections ----
        with tc.tile_pool(name="ps2", bufs=3, space="PSUM") as ps2:
            for c in range(NB):
                cs = slice(c * BK, (c + 1) * BK)
                pq = ps2.tile([P, BK], f32, tag="blk")
                for j in range(EC):
                    nc.tensor.matmul(
                        pq,
                        lhsT=wqk_sb[:, j],
                        rhs=xT_sb[:, j, cs],
                        start=(j == 0),
                        stop=(j == EC - 1),
                    )
                nc.vector.tensor_copy(out=qkT_sb[:, cs], in_=pq)
                pv = ps2.tile([P, BK], f32, tag="blk")
                for j in range(EC):
                    nc.tensor.matmul(
                        pv[:D],
                        lhsT=wv_sb[:, j],
                        rhs=xT_sb[:, j, cs],
                        start=(j == 0),
                        stop=(j == EC - 1),
                    )
                nc.vector.tensor_copy(out=vT_sb[:, cs], in_=pv[:D])

            # kT copy down to base partition 0 (SBUF->SBUF DMA shifts partitions)
            nc.sync.dma_start(kT_sb[:], qkT_sb[D:P, :])

            # v tiles [seq, d] via PE transpose of vT
            for i in range(NT):
                tpb = ps2.tile([P, D], bf16, tag="vt")
                nc.tensor.transpose(
                    tpb, vT_sb[:, i * P : (i + 1) * P], ident_b[:D, :D]
                )
                nc.vector.tensor_copy(out=v_sb[:, i], in_=tpb)

        # ---- Phase A: scores [q, k], softmax, attn output ----
        with tc.tile_pool(name="psA", bufs=2, space="PSUM") as psA:
            for i in range(NT):
                kq = (i + 1) * P if causal else SEQ
                qs = slice(i * P, (i + 1) * P)
                sc = psA.tile([P, SEQ], f32, tag="S")
                for c in range((kq + BK - 1) // BK):
                    n = min(BK, kq - c * BK)
                    nc.tensor.matmul(
                        sc[:, c * BK : c * BK + n],
                        lhsT=qkT_sb[0:D, qs],
                        rhs=kT_sb[:, c * BK : c * BK + n],
                        start=True,
                        stop=True,
                    )
                if causal:
                    nc.vector.tensor_tensor(
                        sc[:, kq - P : kq], sc[:, kq - P : kq], cmask, add
                    )
                e_i = epool.tile([P, SEQ], f32, tag="E")
                rsum = opool.tile([P, 1], f32, tag="rsum")
                nc.scalar.activation(
                    e_i[:, :kq], sc[:, :kq], Exp, scale=0.125, accum_out=rsum
                )
                nc.vector.reciprocal(recip[:, i : i + 1], rsum)
                nc.vector.tensor_scalar_mul(
                    e_i[:, :kq], e_i[:, :kq], recip[:, i : i + 1]
                )
                nc.sync.dma_start(attn[qs, 0:kq], e_i[:, :kq])

        # ---- Phase B: scores [k, q], exp, attn @ v -> outT ----
        with tc.tile_pool(name="psB", bufs=1, space="PSUM") as psB:
            outT_ps = psB.tile([D, SEQ], f32, tag="outT")
            for j in range(NT):
                # st/et columns are q-aligned so every matmul output stays
                # inside one PSUM bank (hardware requirement).
                q0 = j * P if causal else 0  # first valid q for this k-tile
                c0 = q0 // BK
                ks = slice(j * P, (j + 1) * P)
                st = psB.tile([P, SEQ], f32, tag="ST")
                for c in range(c0, NB):
                    cs = slice(c * BK, (c + 1) * BK)
                    nc.tensor.matmul(
                        st[:, cs],
                        lhsT=kT_sb[:, ks],
                        rhs=qkT_sb[0:D, cs],
                        start=True,
                        stop=True,
                    )
                if causal:
                    nc.vector.tensor_tensor(
                        st[:, q0 : q0 + P], st[:, q0 : q0 + P], cmaskT, add
                    )
                et_j = etpool.tile([P, SEQ], bf16, tag="ET")
                if causal and q0 > c0 * BK:
                    nc.vector.memset(et_j[:, c0 * BK : q0], 0.0)
                nc.scalar.activation(
                    et_j[:, q0:], st[:, q0:], Exp, scale=0.125
                )
                for c in range(c0, NB):
                    cs = slice(c * BK, (c + 1) * BK)
                    jlast = min(4 * c + 3, NT - 1) if causal else NT - 1
                    nc.tensor.matmul(
                        outT_ps[:, cs],
                        lhsT=v_sb[:, j],
                        rhs=et_j[:, cs],
                        start=(j == 0),
                        stop=(j == jlast),
                    )
            nc.vector.tensor_copy(out=outT_sb[:], in_=outT_ps[:])

        # ---- Phase 5: outT -> out (transpose + normalize) ----
        with tc.tile_pool(name="psC", bufs=3, space="PSUM") as psC:
            for i in range(NT):
                tp = psC.tile([P, D], f32, tag="tp")
                nc.tensor.transpose(
                    tp, outT_sb[:, i * P : (i + 1) * P], ident_f[:D, :D]
                )
                o_i = opool.tile([P, D], f32, tag="o")
                nc.vector.tensor_scalar_mul(o_i, tp, recip[:, i : i + 1])
                nc.sync.dma_start(out[i * P : (i + 1) * P, :], o_i)


def _get_program(causal: bool):
    if causal not in _programs:
        _programs[causal] = _build_program(causal)
    return _programs[causal]


def _make_in_maps(x, W_q, W_k, W_v):
    import ml_dtypes

    bf16 = ml_dtypes.bfloat16
    x = np.asarray(x, dtype=np.float32)
    W_q = np.asarray(W_q, dtype=np.float32)
    W_k = np.asarray(W_k, dtype=np.float32)
    W_v = np.asarray(W_v, dtype=np.float32)
    assert x.shape == (BATCH, SEQ, EMB), x.shape
    wqk = np.ascontiguousarray(np.concatenate([W_q, W_k], axis=1).astype(bf16))
    wv = np.ascontiguousarray(W_v.astype(bf16))
    return [
        {
            "xT": np.ascontiguousarray(x[b].T.astype(bf16)),
            "wqk": wqk,
            "wv": wv,
        }
        for b in range(BATCH)
    ]


def _run(x, W_q, W_k, W_v, mask, trace=False):
    from concourse.bass_utils import run_bass_kernel_spmd

    causal = bool(np.asarray(mask).item()) if mask is not None else True
    nc = _get_program(causal)
    in_maps = _make_in_maps(x, W_q, W_k, W_v)
    res = run_bass_kernel_spmd(nc, in_maps, core_ids=list(range(BATCH)), trace=trace)
    out = np.stack([np.asarray(r["out"]) for r in res.results], axis=0)
    attn = np.stack([np.asarray(r["attn"]) for r in res.results], axis=0)
    return (out, attn), res


def kernel(x=None, W_q=None, W_k=None, W_v=None, mask=None, **_ignored):
    (out, attn), _ = _run(x, W_q, W_k, W_v, mask)
    return out, attn
